# revision 9
# baseline (speedup 1.0000x reference)
# Trainium2 Bass kernel for: ConvTranspose2d(64->128, k=4, stride=1) -> spatial
# mean -> +biases -> 10*logsumexp over channels.
#
# Math: with full (K-1) output padding, the mean over the ENTIRE conv-transpose
# output spatial extent sees every input pixel through all K*K taps, so
#   pooled[n,co] = (sum_hw x[n,ci,hw]) @ (sum_kk w[ci,co,kk]) / (Ho*Wo) + cb + eb
# exactly. The conv collapses to a spatial sum + a (Cin x Cout) matmul.
#
# Sharding: data-parallel over batch N=32 across 8 cores (4 batches/core).
# The (Cin,Cout) tap-sum of the replicated weight is precomputed on the host
# (param preprocessing, like weight repacking), so each core only streams its
# 4 MiB x-slice plus a 64 KiB param matrix.
#
# Per-core dataflow (trace-driven, see test.py profiling):
# - x arrives as [256, 4096] (row = (n,ci)); each 128-row block is cut into
#   column chunks [1152, 1152, 1152, 640] and the 8 chunk DMAs alternate over
#   BOTH HWDGE rings (SP ring qSyncDynamicHW / ACT ring qScalarDynamicHW); two
#   descriptor streams keep all 16 SDMA engines fed (~350 GB/s, HBM-limited).
# - Per-chunk spatial sums are split across TWO engines: ACT reduces the
#   ACT-ring chunks via activation(Copy, accum_out), DVE the SP-ring ones
#   (either engine alone would trail the DMA stream). The last chunk per row
#   block is the small 640-col one so the reduce trailing the stream is short.
# - Row-block 0 finishes streaming first: its combine + masked copies + fp32r
#   matmul (accumulating into PSUM on top of an early bias matmul) all hide
#   under row-block 1's stream; only row-block 1's short chain trails the DMA.
# - The masked-lhsT trick: weight sums arrive pre-duplicated onto both
#   partition halves and each batch's sums land masked into its own columns,
#   so one 128-deep matmul per row block contracts both its batches.
# - exp-accumulate + log + 10x on ACT, one table set (Exp+Ln+Copy) preloaded
#   at kernel start so no ACT_TABLE_LOAD lands anywhere.

import os

import numpy as np

import concourse.bacc as bacc
import concourse.bass as bass
import concourse.mybir as mybir
import concourse.tile as tile
from concourse.bass_utils import run_bass_kernel_spmd
from concourse.hw_specs import get_activation_tables

N, CIN, COUT, K, H, W = 32, 64, 128, 4, 64, 64
NCORES = 8
NLOC = N // NCORES          # 4 batches per core
HW = H * W                  # 4096
ROWS = NLOC * CIN           # 256 rows (n,ci) per core
RBLK = ROWS // 128          # 2 row blocks of 128 partitions
COLSPLITS = [1152, 1152, 1152, 640]   # tapered: short reduce trails the stream
NCH = len(COLSPLITS)
NCHUNKS = RBLK * NCH        # 8
SCALE = 1.0 / float((H + K - 1) * (W + K - 1))   # 1/4489

F32 = mybir.dt.float32
F32R = mybir.dt.float32r

_CACHE: dict = {}


def _build_module() -> bacc.Bacc:
    nc = bacc.Bacc("TRN2", target_bir_lowering=False, enable_partition_id=False)

    x_d = nc.dram_tensor("xc", [ROWS, HW], F32, kind="ExternalInput").ap()
    w_d = nc.dram_tensor("wsum", [128, COUT], F32R, kind="ExternalInput").ap()
    bs_d = nc.dram_tensor("bs", [2, COUT], F32R, kind="ExternalInput").ap()
    z_d = nc.dram_tensor("zm", [128, RBLK * NLOC], F32R, kind="ExternalInput").ap()
    o_d = nc.dram_tensor("ones", [2, NLOC], F32R, kind="ExternalInput").ap()
    y_d = nc.dram_tensor("y", [NLOC, 1], F32, kind="ExternalOutput").ap()

    with tile.TileContext(nc) as tc:
        with (
            tc.tile_pool(name="xpool", bufs=NCHUNKS) as xpool,
            tc.tile_pool(name="spool", bufs=2) as spool,
            tc.tile_pool(name="small", bufs=1) as small,
            tc.tile_pool(name="psum", bufs=1, space="PSUM") as psum_pool,
        ):
            # preload the one ACT table set that covers Exp, Ln AND Copy
            # ("natural_log_exp_and_others") so no ACT_TABLE_LOAD is inserted
            # anywhere in the chain.
            act_tables = get_activation_tables(nc.m.arch)
            set_id = next(
                i
                for i, (_, funcs) in enumerate(act_tables.items())
                if mybir.ActivationFunctionType.Exp in funcs
                and mybir.ActivationFunctionType.Ln in funcs
                and mybir.ActivationFunctionType.Copy in funcs
            )
            nc.scalar.add_instruction(
                mybir.InstLoadActFuncSet(
                    name=nc.get_next_instruction_name(), act_func_set_id=set_id
                )
            )

            parts = small.tile([128, NCHUNKS], F32)
            wdup = small.tile([128, COUT], F32R)
            biasrows = small.tile([2, COUT], F32R)
            onesb = small.tile([2, NLOC], F32R)
            # one masked lhsT per row block: only its own two columns are ever
            # written, so each accumulating matmul adds zeros to the other
            # rows. fp32r (single-pass PE) demands f32r-typed producers, so
            # the zero mask is DMA'd from DRAM (DVE memset can't write f32r).
            s2m = small.tile([128, RBLK * NLOC], F32R)
            s2 = small.tile([128, RBLK], F32)
            scratch = [
                spool.tile([128, max(COLSPLITS)], F32, name=f"scratch{j}")
                for j in range(2)
            ]


            # ---- x chunk DMAs: row-block-major so rb0 completes first; rings
            # alternate A,S,A,S per block so the last (small) chunk of each
            # block lands on DVE, keeping the tail chain on one engine.
            cols = []
            off = 0
            for w_ in COLSPLITS:
                cols.append((off, w_))
                off += w_
            xts = []
            for i in range(NCHUNKS):
                rb, c = i // NCH, i % NCH
                o, w_ = cols[c]
                xt = xpool.tile([128, w_], F32, tag=f"xt{c}")
                eng = nc.scalar if c % 2 == 0 else nc.sync
                eng.dma_start(
                    out=xt, in_=x_d[rb * 128 : (rb + 1) * 128, o : o + w_]
                )
                xts.append((rb, c, xt))

            # params ride the ACT ring after the x chunks (needed only mid-
            # kernel, and their stream time is tiny).
            nc.scalar.dma_start(out=wdup, in_=w_d)
            nc.scalar.dma_start(out=biasrows, in_=bs_d)
            nc.scalar.dma_start(out=s2m, in_=z_d)
            nc.scalar.dma_start(out=onesb, in_=o_d)

            # ---- early bias matmul opens the PSUM accumulation group ----
            pooled = psum_pool.tile([NLOC, COUT], F32, space="PSUM")
            nc.tensor.matmul(
                out=pooled, lhsT=onesb, rhs=biasrows, start=True, stop=False
            )

            # ---- per-chunk spatial sums (ACT for even c, DVE for odd c),
            # then per-row-block combine + masked copies + fp32r matmul ----
            for i, (rb, c, xt) in enumerate(xts):
                if c % 2 == 0:
                    nc.scalar.activation(
                        out=scratch[c // 2][:, 0 : xt.shape[1]],
                        in_=xt,
                        func=mybir.ActivationFunctionType.Copy,
                        accum_out=parts[:, i : i + 1],
                    )
                else:
                    nc.vector.reduce_sum(
                        out=parts[:, i : i + 1], in_=xt, axis=mybir.AxisListType.X
                    )
                if c == NCH - 1:
                    # all partials of this row block are in parts[:, rb*NCH:...]
                    nc.vector.reduce_sum(
                        out=s2[:, rb : rb + 1],
                        in_=parts.rearrange("p (r c) -> p r c", r=RBLK)[
                            :, rb : rb + 1, :
                        ],
                        axis=mybir.AxisListType.X,
                    )
                    # masked copies: s2m_rb[(n%2)*64+ci, n] = s2[.., rb],
                    # n = 2rb+half; the other block's columns stay zero
                    base = rb * NLOC
                    nc.vector.tensor_copy(
                        s2m[0:64, base + 2 * rb : base + 2 * rb + 1],
                        s2[0:64, rb : rb + 1],
                    )
                    nc.vector.tensor_copy(
                        s2m[64:128, base + 2 * rb + 1 : base + 2 * rb + 2],
                        s2[64:128, rb : rb + 1],
                    )
                    # accumulate this row block's two batches into PSUM
                    nc.tensor.matmul(
                        out=pooled,
                        lhsT=s2m[:, base : base + NLOC],
                        rhs=wdup,
                        start=False,
                        stop=(rb == RBLK - 1),
                        skip_group_check=True,
                    )

            # ---- 10 * log(sum_co exp(pooled)) ----
            expt = small.tile([NLOC, COUT], F32)
            sume = small.tile([NLOC, 1], F32)
            nc.scalar.activation(
                out=expt,
                in_=pooled,
                func=mybir.ActivationFunctionType.Exp,
                accum_out=sume,
            )
            logv = small.tile([NLOC, 1], F32)
            nc.scalar.activation(
                out=logv, in_=sume, func=mybir.ActivationFunctionType.Ln
            )
            outv = small.tile([NLOC, 1], F32)
            nc.scalar.mul(out=outv, in_=logv, mul=10.0)
            nc.scalar.dma_start(out=y_d, in_=outv)

    nc.compile()
    return nc


def kernel(x, weight, conv_bias, extra_bias):
    x = np.ascontiguousarray(np.asarray(x, dtype=np.float32))
    weight = np.ascontiguousarray(np.asarray(weight, dtype=np.float32))
    conv_bias = np.ascontiguousarray(np.asarray(conv_bias, dtype=np.float32))
    extra_bias = np.ascontiguousarray(np.asarray(extra_bias, dtype=np.float32))
    assert x.shape == (N, CIN, H, W), x.shape
    assert weight.shape == (CIN, COUT, K, K), weight.shape

    if "nc" not in _CACHE:
        _CACHE["nc"] = _build_module()
    nc = _CACHE["nc"]

    # host-side param preprocessing: scaled tap-sum, duplicated onto both
    # partition halves so each batch contracts against its own half.
    ws = (weight.reshape(CIN, COUT, K * K).sum(axis=2) * SCALE).astype(np.float32)
    wdup = np.ascontiguousarray(np.vstack([ws, ws]))  # (128, COUT)
    bs2 = np.ascontiguousarray(
        np.stack([conv_bias, extra_bias], axis=0)
    )  # (2, COUT)
    zm = np.zeros((128, RBLK * NLOC), dtype=np.float32)
    ones = np.ones((2, NLOC), dtype=np.float32)
    in_maps = []
    for c in range(NCORES):
        xc = x[c * NLOC : (c + 1) * NLOC].reshape(ROWS, HW)
        in_maps.append(
            {"xc": xc, "wsum": wdup, "bs": bs2, "zm": zm, "ones": ones}
        )

    trace = os.environ.get("BASS_KERNEL_TRACE") == "1"
    res = run_bass_kernel_spmd(
        nc, in_maps, core_ids=list(range(NCORES)), trace=trace
    )
    _CACHE["last_result"] = res
    return np.concatenate([r["y"] for r in res.results], axis=0)


# revision 11
# speedup vs baseline: 1.1038x; 1.1038x over previous
# Trainium2 Bass kernel for: ConvTranspose2d(64->128, k=4, stride=1) -> spatial
# mean -> +biases -> 10*logsumexp over channels.
#
# Math: with full (K-1) output padding, the mean over the ENTIRE conv-transpose
# output spatial extent sees every input pixel through all K*K taps, so
#   pooled[n,co] = (sum_hw x[n,ci,hw]) @ (sum_kk w[ci,co,kk]) / (Ho*Wo) + cb + eb
# exactly. The conv collapses to a spatial sum + a (Cin x Cout) matmul.
#
# Sharding: data-parallel over batch N=32 across 8 cores (4 batches/core).
# The (Cin,Cout) tap-sum of the replicated weight is precomputed on the host
# (param preprocessing, like weight repacking), so each core only streams its
# 4 MiB x-slice plus a 64 KiB param matrix.
#
# Per-core dataflow (trace-driven, see test.py profiling):
# - x arrives as [256, 4096] (row = (n,ci)); each 128-row block is cut into
#   column chunks [1152, 1152, 1152, 640] and the 8 chunk DMAs alternate over
#   BOTH HWDGE rings (SP ring qSyncDynamicHW / ACT ring qScalarDynamicHW); two
#   descriptor streams keep all 16 SDMA engines fed (~350 GB/s, HBM-limited).
# - Per-chunk spatial sums are split across TWO engines: ACT reduces the
#   ACT-ring chunks via activation(Copy, accum_out), DVE the SP-ring ones
#   (either engine alone would trail the DMA stream). The last chunk per row
#   block is the small 640-col one so the reduce trailing the stream is short.
# - Row-block 0 finishes streaming first: its combine + masked copies + fp32r
#   matmul (accumulating into PSUM on top of an early bias matmul) all hide
#   under row-block 1's stream; only row-block 1's short chain trails the DMA.
# - The masked-lhsT trick: weight sums arrive pre-duplicated onto both
#   partition halves and each batch's sums land masked into its own columns,
#   so one 128-deep matmul per row block contracts both its batches.
# - exp-accumulate + log + 10x on ACT, one table set (Exp+Ln+Copy) preloaded
#   at kernel start so no ACT_TABLE_LOAD lands anywhere.

import os

import numpy as np

import concourse.bacc as bacc
import concourse.bass as bass
import concourse.mybir as mybir
import concourse.tile as tile
from concourse.bass_utils import run_bass_kernel_spmd
from concourse.hw_specs import get_activation_tables

N, CIN, COUT, K, H, W = 32, 64, 128, 4, 64, 64
NCORES = 8
NLOC = N // NCORES          # 4 batches per core
HW = H * W                  # 4096
ROWS = NLOC * CIN           # 256 rows (n,ci) per core
RBLK = ROWS // 128          # 2 row blocks of 128 partitions
COLSPLITS = [1152, 1152, 1152, 640]   # tapered: short reduce trails the stream
NCH = len(COLSPLITS)
NCHUNKS = RBLK * NCH        # 8
SCALE = 1.0 / float((H + K - 1) * (W + K - 1))   # 1/4489

F32 = mybir.dt.float32
F32R = mybir.dt.float32r

_CACHE: dict = {}


def _build_module() -> bacc.Bacc:
    nc = bacc.Bacc("TRN2", target_bir_lowering=False, enable_partition_id=False)

    x_d = nc.dram_tensor("xc", [ROWS, HW], F32, kind="ExternalInput").ap()
    w_d = nc.dram_tensor("wsum", [128, COUT], F32R, kind="ExternalInput").ap()
    bs_d = nc.dram_tensor("bs", [2, COUT], F32R, kind="ExternalInput").ap()
    z_d = nc.dram_tensor("zm", [128, RBLK * NLOC], F32R, kind="ExternalInput").ap()
    o_d = nc.dram_tensor("ones", [2, NLOC], F32R, kind="ExternalInput").ap()
    y_d = nc.dram_tensor("y", [NLOC, 1], F32, kind="ExternalOutput").ap()

    with tile.TileContext(nc) as tc:
        with (
            tc.tile_pool(name="xpool", bufs=NCHUNKS) as xpool,
            tc.tile_pool(name="spool", bufs=2) as spool,
            tc.tile_pool(name="small", bufs=1) as small,
            tc.tile_pool(name="psum", bufs=1, space="PSUM") as psum_pool,
        ):
            # preload the one ACT table set that covers Exp, Ln AND Copy
            # ("natural_log_exp_and_others") so no ACT_TABLE_LOAD is inserted
            # anywhere in the chain.
            act_tables = get_activation_tables(nc.m.arch)
            set_id = next(
                i
                for i, (_, funcs) in enumerate(act_tables.items())
                if mybir.ActivationFunctionType.Exp in funcs
                and mybir.ActivationFunctionType.Ln in funcs
                and mybir.ActivationFunctionType.Copy in funcs
            )
            nc.scalar.add_instruction(
                mybir.InstLoadActFuncSet(
                    name=nc.get_next_instruction_name(), act_func_set_id=set_id
                )
            )

            parts = small.tile([128, NCHUNKS], F32)
            wdup = small.tile([128, COUT], F32R)
            biasrows = small.tile([2, COUT], F32R)
            onesb = small.tile([2, NLOC], F32R)
            # one masked lhsT per row block: only its own two columns are ever
            # written, so each accumulating matmul adds zeros to the other
            # rows. fp32r (single-pass PE) demands f32r-typed producers, so
            # the zero mask is DMA'd from DRAM (DVE memset can't write f32r).
            s2m = small.tile([128, RBLK * NLOC], F32R)
            s2 = small.tile([128, RBLK], F32)
            scratch = [
                spool.tile([128, max(COLSPLITS)], F32, name=f"scratch{j}")
                for j in range(2)
            ]


            # params ride the GpSimd SWDGE queue, issued FIRST: the engine is
            # otherwise idle, so these DMAs never block the ACT/SP HWDGE
            # streams and grab fresh semaphore lanes.
            nc.gpsimd.dma_start(out=wdup, in_=w_d)
            nc.gpsimd.dma_start(out=biasrows, in_=bs_d)
            nc.gpsimd.dma_start(out=s2m, in_=z_d)
            nc.gpsimd.dma_start(out=onesb, in_=o_d)

            # ---- x chunk DMAs: row-block-major so rb0 completes first; rings
            # alternate A,S,A,S per block so the last (small) chunk of each
            # block lands on DVE, keeping the tail chain on one engine.
            cols = []
            off = 0
            for w_ in COLSPLITS:
                cols.append((off, w_))
                off += w_
            xts = []
            for i in range(NCHUNKS):
                rb, c = i // NCH, i % NCH
                o, w_ = cols[c]
                xt = xpool.tile([128, w_], F32, tag=f"xt{c}")
                eng = nc.scalar if c % 2 == 0 else nc.sync
                eng.dma_start(
                    out=xt, in_=x_d[rb * 128 : (rb + 1) * 128, o : o + w_]
                )
                xts.append((rb, c, xt))

            # ---- early bias matmul opens the PSUM accumulation group ----
            pooled = psum_pool.tile([NLOC, COUT], F32, space="PSUM")
            nc.tensor.matmul(
                out=pooled, lhsT=onesb, rhs=biasrows, start=True, stop=False
            )

            # ---- per-chunk spatial sums (ACT for even c, DVE for odd c),
            # then per-row-block combine + masked copies + fp32r matmul ----
            for i, (rb, c, xt) in enumerate(xts):
                if c % 2 == 0:
                    nc.scalar.activation(
                        out=scratch[c // 2][:, 0 : xt.shape[1]],
                        in_=xt,
                        func=mybir.ActivationFunctionType.Copy,
                        accum_out=parts[:, i : i + 1],
                    )
                else:
                    nc.vector.reduce_sum(
                        out=parts[:, i : i + 1], in_=xt, axis=mybir.AxisListType.X
                    )
                if c == NCH - 1:
                    # all partials of this row block are in parts[:, rb*NCH:...]
                    nc.vector.reduce_sum(
                        out=s2[:, rb : rb + 1],
                        in_=parts.rearrange("p (r c) -> p r c", r=RBLK)[
                            :, rb : rb + 1, :
                        ],
                        axis=mybir.AxisListType.X,
                    )
                    # masked copies: s2m_rb[(n%2)*64+ci, n] = s2[.., rb],
                    # n = 2rb+half; the other block's columns stay zero
                    base = rb * NLOC
                    nc.vector.tensor_copy(
                        s2m[0:64, base + 2 * rb : base + 2 * rb + 1],
                        s2[0:64, rb : rb + 1],
                    )
                    nc.vector.tensor_copy(
                        s2m[64:128, base + 2 * rb + 1 : base + 2 * rb + 2],
                        s2[64:128, rb : rb + 1],
                    )
                    # accumulate this row block's two batches into PSUM
                    nc.tensor.matmul(
                        out=pooled,
                        lhsT=s2m[:, base : base + NLOC],
                        rhs=wdup,
                        start=False,
                        stop=(rb == RBLK - 1),
                        skip_group_check=True,
                    )

            # ---- 10 * log(sum_co exp(pooled)) ----
            expt = small.tile([NLOC, COUT], F32)
            sume = small.tile([NLOC, 1], F32)
            nc.scalar.activation(
                out=expt,
                in_=pooled,
                func=mybir.ActivationFunctionType.Exp,
                accum_out=sume,
            )
            logv = small.tile([NLOC, 1], F32)
            nc.scalar.activation(
                out=logv, in_=sume, func=mybir.ActivationFunctionType.Ln
            )
            outv = small.tile([NLOC, 1], F32)
            nc.scalar.mul(out=outv, in_=logv, mul=10.0)
            nc.scalar.dma_start(out=y_d, in_=outv)

    nc.compile()
    return nc


def kernel(x, weight, conv_bias, extra_bias):
    x = np.ascontiguousarray(np.asarray(x, dtype=np.float32))
    weight = np.ascontiguousarray(np.asarray(weight, dtype=np.float32))
    conv_bias = np.ascontiguousarray(np.asarray(conv_bias, dtype=np.float32))
    extra_bias = np.ascontiguousarray(np.asarray(extra_bias, dtype=np.float32))
    assert x.shape == (N, CIN, H, W), x.shape
    assert weight.shape == (CIN, COUT, K, K), weight.shape

    if "nc" not in _CACHE:
        _CACHE["nc"] = _build_module()
    nc = _CACHE["nc"]

    # host-side param preprocessing: scaled tap-sum, duplicated onto both
    # partition halves so each batch contracts against its own half.
    ws = (weight.reshape(CIN, COUT, K * K).sum(axis=2) * SCALE).astype(np.float32)
    wdup = np.ascontiguousarray(np.vstack([ws, ws]))  # (128, COUT)
    bs2 = np.ascontiguousarray(
        np.stack([conv_bias, extra_bias], axis=0)
    )  # (2, COUT)
    zm = np.zeros((128, RBLK * NLOC), dtype=np.float32)
    ones = np.ones((2, NLOC), dtype=np.float32)
    in_maps = []
    for c in range(NCORES):
        xc = x[c * NLOC : (c + 1) * NLOC].reshape(ROWS, HW)
        in_maps.append(
            {"xc": xc, "wsum": wdup, "bs": bs2, "zm": zm, "ones": ones}
        )

    trace = os.environ.get("BASS_KERNEL_TRACE") == "1"
    res = run_bass_kernel_spmd(
        nc, in_maps, core_ids=list(range(NCORES)), trace=trace
    )
    _CACHE["last_result"] = res
    return np.concatenate([r["y"] for r in res.results], axis=0)
